# revision 13
# baseline (speedup 1.0000x reference)
"""Causal attention (B=4, S=4096, D_IN=768, D_OUT=64) on 8 Trainium2 NeuronCores.

Sharding: core c handles batch b=c//2 and key-parity p=c%2 (the even or odd
128-wide key tiles of that batch). Every core computes, for ALL queries of its
batch, the unnormalized attention partials over its own key set:
    num[o, q] = sum_{k in own} exp(q.k/8) * V[k, o]
    den[q]    = sum_{k in own} exp(q.k/8)
The host sums the two partials per batch and normalizes: ctx = (num/den).T.
Causality is exact: key-tile work is skipped below the diagonal band and the
boundary blocks are masked with host-provided mask tiles.

Pipeline design (all bf16 on-chip, fp32 PSUM):
 - The exp (Scalar/ACT engine) is the critical resource (~35us of streaming);
   everything is scheduled around keeping it 100% fed:
   * query tiles are processed in the order 0,1,3,7,2,6,5,4 - light tiles
     first (fast first-exp), heavy tiles in the middle (ACT saturated),
     light tiles last (small drain tail). x DMA is issued in the matching
     column order.
   * scores land in 3-bank PSUM tiles and are exp'd 1536 columns per ACT
     instruction; the whole 4-tile diagonal band shares one 3-bank tile and
     one ACT.
   * projection matmuls for later column blocks are emitted in small chunks
     between scores groups so the PE never parks ACT-feeding work behind a
     monolithic projection block.
 - Scores matmuls run as CONCURRENT K=64 pairs on disjoint PE row-groups
   (kp[i][0:64] = K^T of key tile 2i, kp[i][64:128] = tile 2i+1; Wq is sent
   duplicated so Q^T exists at partitions 0..63 AND 64..127).
 - ctx matmuls stay M=65 (64 V columns + ones column -> denominator free).
 - A few dummy matmuls at t=0 warm the PE HAM clock-gate while x DMA lands.
"""
import numpy as np

import concourse.bass as bass
import concourse.bacc as bacc
import concourse.tile as tile
from concourse import mybir
from concourse.bass_utils import run_bass_kernel_spmd

B, S, DI, DO = 4, 4096, 768, 64
NCORES = 8
NIC = DI // 128          # 6 contraction chunks
NKT = S // 128           # 32 global key tiles per batch
NOWN = NKT // 2          # 16 own key tiles per core
QT = 512                 # query tile width
NQT = S // QT            # 8 query tiles
ORD = [0, 1, 3, 7, 2, 6, 5, 4]       # query-tile processing order
XORD = [0, 1, 3, 2, 7, 4, 5, 6]      # x column-block DMA order
F32 = mybir.dt.float32
BF16 = mybir.dt.bfloat16

_prog_cache = {}


def j0_of(T):
    """First diagonal-region packed key tile for permuted query tile T."""
    return 4 * T if T < 4 else 4 * (T - 4)


def build_program():
    """Build + compile the single SPMD Bass program (identical on all cores)."""
    nc = bacc.Bacc("TRN2", target_bir_lowering=False, debug=False)

    xT = nc.declare_dram_parameter("xT", [DI, S], BF16, isOutput=False)
    wkv = nc.declare_dram_parameter("wkv", [DI, 128], BF16, isOutput=False)
    # [Wq | Wq] duplicated so Q^T appears at partitions 0..63 AND 64..127,
    # feeding the two concurrent K=64 row-group matmuls of a scores pair.
    wq2 = nc.declare_dram_parameter("wq2", [DI, 128], BF16, isOutput=False)
    mdiag = nc.declare_dram_parameter("mdiag", [128, 128], BF16, isOutput=False)
    mpcol = nc.declare_dram_parameter("mpcol", [128, 128], BF16, isOutput=False)
    ident = nc.declare_dram_parameter("ident", [DO, DO], BF16, isOutput=False)
    nd = nc.declare_dram_parameter("nd", [DO + 1, S], F32, isOutput=True)

    with tile.TileContext(nc) as tc:
        with tc.tile_pool(name="consts", bufs=1) as consts, \
             tc.tile_pool(name="xpool", bufs=1) as xpool, \
             tc.tile_pool(name="qkv", bufs=1) as qkv, \
             tc.tile_pool(name="expp", bufs=10) as expp, \
             tc.tile_pool(name="ndst", bufs=2) as ndst, \
             tc.tile_pool(name="ps_sc", bufs=2, space="PSUM") as ps_sc, \
             tc.tile_pool(name="ps_pj", bufs=1, space="PSUM") as ps_pj, \
             tc.tile_pool(name="ps_ctx", bufs=1, space="PSUM") as ps_ctx:

            # ---- HAM warm-up: keep the PE busy from t=0 while DMA lands, so
            # the clock-gate reaches K=8/8 before the real work arrives.
            dum = consts.tile([128, 128], BF16, tag="dum", name="dum")
            nc.vector.memset(dum, 0.0)
            pdum = ps_sc.tile([128, 3 * QT], F32, tag="psc", name="psc")
            for _ in range(12):
                nc.tensor.matmul(pdum[:, 0:128], dum, dum, start=True, stop=True)

            # ---- constant-ish inputs; the first projection's weights go first,
            # then the first x column block, so the PE can start ASAP
            twkv = consts.tile([128, NIC, 128], BF16, tag="twkv", name="twkv")
            twq = consts.tile([128, NIC, 128], BF16, tag="twq", name="twq")
            nc.sync.dma_start(out=twkv, in_=wkv.rearrange("(c p) w -> p c w", p=128))

            # x^T column blocks, ONE [128, 6, 512] DMA each (6KB/partition -
            # large transfers amortize the ~per-DMA completion latency),
            # issued in XORD order
            xb = [xpool.tile([128, NIC, QT], BF16, tag=f"xb_{cb}", name=f"xb_{cb}")
                  for cb in range(NQT)]
            nc.sync.dma_start(
                out=xb[0],
                in_=xT[:, 0:QT].rearrange("(c p) w -> p c w", p=128))

            nc.sync.dma_start(out=twq, in_=wq2.rearrange("(c p) w -> p c w", p=128))
            tmd = consts.tile([128, 128], BF16, tag="tmd", name="tmd")
            tmp = consts.tile([128, 128], BF16, tag="tmp", name="tmp")
            tid = consts.tile([DO, DO], BF16, tag="tid", name="tid")
            nc.sync.dma_start(out=tmd, in_=mdiag[:, :])
            nc.sync.dma_start(out=tmp, in_=mpcol[:, :])
            nc.sync.dma_start(out=tid, in_=ident[:, :])
            zsrc = consts.tile([DO, 1], F32, tag="zsrc", name="zsrc")
            nc.vector.memset(zsrc, 0.0)

            # Dummy exp to pull the ~2.7us ACT table load off the critical path.
            zexp = consts.tile([DO, 1], F32, tag="zexp", name="zexp")
            nc.scalar.activation(zexp, zsrc,
                                 mybir.ActivationFunctionType.Exp, scale=1.0)

            for cb in XORD[1:]:
                nc.sync.dma_start(
                    out=xb[cb],
                    in_=xT[:, cb * QT:(cb + 1) * QT].rearrange(
                        "(c p) w -> p c w", p=128))

            def xc(ic, cb):
                return xb[cb][:, ic, :]

            # ---- projection state (emitted interleaved with attention) ----
            # kp[i]: K^T of key tile 2i at partitions 0..63, tile 2i+1 at 64..127
            kps = [qkv.tile([128, 128], BF16, tag=f"kp_{i}", name=f"kp_{i}")
                   for i in range(NOWN // 2)]
            vts = [qkv.tile([DO, QT], BF16, tag=f"vt_{st}", name=f"vt_{st}") for st in range(4)]
            qts = [qkv.tile([128, QT], BF16, tag=f"qt_{st}", name=f"qt_{st}") for st in range(NQT)]
            # all V1 tiles in one buffer: [128 keys, key tile, 64 V cols + ones]
            v1big = qkv.tile([128, NOWN, DO + 1], BF16, tag="v1big", name="v1big")
            nc.vector.memset(v1big[:, :, DO:DO + 1], 1.0)

            def v1(j):
                return v1big[:, j, :]

            def pass1_units(st):
                """Small PE work units for the K/V projection of own key
                column block st (emitted interleaved with attention)."""
                p1 = ps_pj.tile([128, QT], F32, tag="pspj", name="pspj")
                for ic in range(0, NIC, 2):
                    def mm2(ic=ic, p1=p1):
                        nc.tensor.matmul(p1, twkv[:, ic, :], xc(ic, st),
                                         start=(ic == 0), stop=False)
                        nc.tensor.matmul(p1, twkv[:, ic + 1, :], xc(ic + 1, st),
                                         start=False, stop=(ic + 1 == NIC - 1))
                    yield mm2

                def copies(p1=p1):
                    nc.vector.tensor_copy(vts[st], p1[DO:128, :])
                    for u in range(2):
                        kp = kps[2 * st + u]
                        nc.vector.tensor_copy(kp[0:DO, :],
                                              p1[0:DO, 256 * u:256 * u + 128])
                        nc.vector.tensor_copy(kp[DO:128, :],
                                              p1[0:DO, 256 * u + 128:256 * u + 256])
                yield copies

                def transp():
                    pvq = ps_pj.tile([128, 4, DO], BF16, tag="pspj", name="pspj")
                    for r in range(4):
                        nc.tensor.transpose(pvq[:, r, :],
                                            vts[st][:, r * 128:r * 128 + 128], tid)
                    nc.vector.tensor_copy(v1big[:, 4 * st:4 * st + 4, 0:DO], pvq)
                yield transp

            def pass2_units(st):
                """Q^T (duplicated at partitions 0..63 / 64..127) for block st."""
                p2 = ps_pj.tile([128, QT], F32, tag="pspj", name="pspj")
                for ic in range(0, NIC, 2):
                    def mm2(ic=ic, p2=p2):
                        nc.tensor.matmul(p2, twq[:, ic, :], xc(ic, st),
                                         start=(ic == 0), stop=False)
                        nc.tensor.matmul(p2, twq[:, ic + 1, :], xc(ic + 1, st),
                                         start=False, stop=(ic + 1 == NIC - 1))
                    yield mm2

                def qcopy(p2=p2):
                    nc.vector.tensor_copy(qts[st], p2)
                yield qcopy

            # projection units consumed during each position of ORD
            PROJ = {0: [(1, True), (1, False)],
                    1: [(3, False), (3, True), (2, True)],
                    2: [(7, False)], 3: [(2, False)], 4: [(6, False)],
                    5: [(5, False)], 6: [(4, False)], 7: []}

            exp_scale = float(1.0 / np.sqrt(DO))

            def mm_sc(T, j, w, sp, off):
                """One K=64 scores matmul: key tile j x last w queries of tile
                T, into sp[:, off:off+w]. Row-group from j's parity."""
                kp = kps[j // 2]
                lo = DO * (j % 2)
                nc.tensor.matmul(sp[:, off:off + w], kp[lo:lo + DO, :],
                                 qts[T][lo:lo + DO, QT - w:QT],
                                 start=True, stop=True)

            class CtxDrain:
                """Phase B for a query tile, drained a few matmuls at a time so
                ready ctx work interleaves between the next tile's scores
                groups in the in-order PE queue."""

                def __init__(self, T, ctx_args):
                    self.T = T
                    self.nk = j0_of(T) + 4
                    self.args = ctx_args
                    self.i = 0
                    self.ctxp = ps_ctx.tile([DO + 1, QT], F32, tag="ctxp",
                                            name="ctxp")

                def drain(self, n):
                    while self.i < len(self.args) and n > 0:
                        j, et_ap, qlo, w = self.args[self.i]
                        nc.tensor.matmul(self.ctxp[:, qlo:QT], v1(j),
                                         et_ap[:, 0:w],
                                         start=(j == 0), stop=(j == self.nk - 1))
                        self.i += 1
                        n -= 1

                def finish(self):
                    self.drain(len(self.args))
                    ost = ndst.tile([DO + 1, QT], F32, tag="ost", name="ost")
                    nc.vector.tensor_copy(ost, self.ctxp)
                    nc.sync.dma_start(out=nd[:, self.T * QT:(self.T + 1) * QT],
                                      in_=ost)

            for u in pass1_units(0):
                u()
            for u in pass2_units(0):
                u()
            pending = None  # CtxDrain from the previous iteration
            for pos in range(NQT):
                T = ORD[pos]
                j0 = j0_of(T)
                mask = tmd if T < 4 else tmp
                ctx_args = []   # (j, et_ap, qlo, w) consumed in phase B

                projq = []      # projection units for upcoming column blocks
                for st, is_p1 in PROJ[pos]:
                    projq.extend(pass1_units(st) if is_p1 else pass2_units(st))
                projq.reverse()  # consume with pop()

                def do_proj(n):
                    while projq and n > 0:
                        projq.pop()()
                        n -= 1

                # full key tiles 0..j0-1 in triples sharing one 3-bank PSUM
                # tile and one 1536-col exp
                ngroups = -(-j0 // 3) + 1
                dn = -(-(len(pending.args) if pending else 0) // ngroups)
                j = 0
                while j < j0:
                    cnt = min(3, j0 - j)
                    sp = ps_sc.tile([128, 3 * QT], F32, tag="psc", name="psc")
                    et = expp.tile([128, 3 * QT], BF16, tag="et", name="et")
                    for u in range(cnt):
                        mm_sc(T, j + u, QT, sp, u * QT)
                    nc.scalar.activation(et[:, 0:cnt * QT], sp[:, 0:cnt * QT],
                                         mybir.ActivationFunctionType.Exp,
                                         scale=exp_scale)
                    for u in range(cnt):
                        ctx_args.append((j + u, et[:, u * QT:(u + 1) * QT], 0, QT))
                    if pending is not None:
                        pending.drain(dn)
                    do_proj(1)
                    j += cnt
                # diagonal band: all 4 tiles in ONE 3-bank tile / one exp:
                # r0 [0:512] bank1, r1 [512:896] bank2, r3 [896:1024] bank2,
                # r2 [1024:1280] bank3 (concurrent pairs hit distinct banks).
                sp = ps_sc.tile([128, 3 * QT], F32, tag="psc", name="psc")
                et = expp.tile([128, 3 * QT], BF16, tag="et", name="et")
                mm_sc(T, j0, QT, sp, 0)
                mm_sc(T, j0 + 1, 384, sp, QT)
                mm_sc(T, j0 + 2, 256, sp, 2 * QT)
                mm_sc(T, j0 + 3, 128, sp, QT + 384)
                nc.scalar.activation(et[:, 0:2 * QT + 256], sp[:, 0:2 * QT + 256],
                                     mybir.ActivationFunctionType.Exp,
                                     scale=exp_scale)
                nc.vector.tensor_mul(et[:, 0:128], et[:, 0:128], mask)
                nc.vector.tensor_mul(et[:, QT:QT + 128], et[:, QT:QT + 128], mask)
                nc.vector.tensor_mul(et[:, QT + 384:2 * QT], et[:, QT + 384:2 * QT],
                                     mask)
                nc.vector.tensor_mul(et[:, 2 * QT:2 * QT + 128],
                                     et[:, 2 * QT:2 * QT + 128], mask)
                ctx_args.append((j0, et[:, 0:QT], 0, QT))
                ctx_args.append((j0 + 1, et[:, QT:QT + 384], 128, 384))
                ctx_args.append((j0 + 2, et[:, 2 * QT:2 * QT + 256], 256, 256))
                ctx_args.append((j0 + 3, et[:, QT + 384:2 * QT], 384, 128))

                if pending is not None:
                    pending.finish()
                do_proj(len(projq))
                pending = CtxDrain(T, ctx_args)
            pending.finish()

    nc.compile()
    return nc


def get_program():
    if "nc" not in _prog_cache:
        _prog_cache["nc"] = build_program()
    return _prog_cache["nc"]


def core_perm(parity):
    """Permuted-to-global column index map: own key tiles first, then other."""
    own = [g for g in range(NKT) if g % 2 == parity]
    other = [g for g in range(NKT) if g % 2 != parity]
    return np.concatenate([np.arange(g * 128, (g + 1) * 128) for g in own + other])


def _to_bf16(a):
    from concourse import mybir as _mybir
    return np.ascontiguousarray(a.astype(_mybir.dt.np(_mybir.dt.bfloat16)))


def make_in_maps(x, Wq, Wk, Wv):
    x = np.asarray(x, dtype=np.float32)
    Wq = np.asarray(Wq, dtype=np.float32)
    Wk = np.asarray(Wk, dtype=np.float32)
    Wv = np.asarray(Wv, dtype=np.float32)
    wkv = _to_bf16(np.concatenate([Wk, Wv], axis=1))
    wq2 = _to_bf16(np.concatenate([Wq, Wq], axis=1))
    mdiag = _to_bf16(np.triu(np.ones((128, 128), dtype=np.float32)))
    ident = _to_bf16(np.eye(DO, dtype=np.float32))
    in_maps = []
    perms = []
    for c in range(NCORES):
        b, par = c // 2, c % 2
        perm = core_perm(par)
        perms.append(perm)
        xTp = _to_bf16(np.ascontiguousarray(x[b].T[:, perm]))
        mpcol = _to_bf16(np.full((128, 128), 1.0 - par, dtype=np.float32))
        in_maps.append({
            "xT": xTp, "wkv": wkv, "wq2": wq2,
            "mdiag": mdiag, "mpcol": mpcol, "ident": ident,
        })
    return in_maps, perms


def combine(results, perms):
    out = np.empty((B, S, DO), dtype=np.float32)
    for b in range(B):
        num = np.zeros((DO, S), dtype=np.float64)
        den = np.zeros((S,), dtype=np.float64)
        for c in (2 * b, 2 * b + 1):
            nd_c = results[c]["nd"].astype(np.float64)
            inv = np.empty(S, dtype=np.int64)
            inv[perms[c]] = np.arange(S)
            nd_g = nd_c[:, inv]
            num += nd_g[:DO]
            den += nd_g[DO]
        out[b] = (num / den).T.astype(np.float32)
    return out


def kernel(x, Wq, Wk, Wv):
    nc = get_program()
    in_maps, perms = make_in_maps(x, Wq, Wk, Wv)
    res = run_bass_kernel_spmd(nc, in_maps, list(range(NCORES)))
    return combine(res.results, perms)


# revision 14
# speedup vs baseline: 1.0261x; 1.0261x over previous
"""Causal attention (B=4, S=4096, D_IN=768, D_OUT=64) on 8 Trainium2 NeuronCores.

Sharding: core c handles batch b=c//2 and key-parity p=c%2 (the even or odd
128-wide key tiles of that batch). Every core computes, for ALL queries of its
batch, the unnormalized attention partials over its own key set:
    num[o, q] = sum_{k in own} exp(q.k/8) * V[k, o]
    den[q]    = sum_{k in own} exp(q.k/8)
The host sums the two partials per batch and normalizes: ctx = (num/den).T.
Causality is exact: key-tile work is skipped below the diagonal band and the
boundary blocks are masked with host-provided mask tiles.

Pipeline design (all bf16 on-chip, fp32 PSUM):
 - The exp (Scalar/ACT engine) is the critical resource (~35us of streaming);
   everything is scheduled around keeping it 100% fed:
   * query tiles are processed in the order 0,1,3,7,2,6,5,4 - light tiles
     first (fast first-exp), heavy tiles in the middle (ACT saturated),
     light tiles last (small drain tail). x DMA is issued in the matching
     column order.
   * scores land in 3-bank PSUM tiles and are exp'd 1536 columns per ACT
     instruction; the whole 4-tile diagonal band shares one 3-bank tile and
     one ACT.
   * projection matmuls for later column blocks are emitted in small chunks
     between scores groups so the PE never parks ACT-feeding work behind a
     monolithic projection block.
 - Scores matmuls run as CONCURRENT K=64 pairs on disjoint PE row-groups
   (kp[i][0:64] = K^T of key tile 2i, kp[i][64:128] = tile 2i+1; Wq is sent
   duplicated so Q^T exists at partitions 0..63 AND 64..127).
 - ctx matmuls stay M=65 (64 V columns + ones column -> denominator free).
 - A few dummy matmuls at t=0 warm the PE HAM clock-gate while x DMA lands.
"""
import numpy as np

import concourse.bass as bass
import concourse.bacc as bacc
import concourse.tile as tile
from concourse import mybir
from concourse.bass_utils import run_bass_kernel_spmd

B, S, DI, DO = 4, 4096, 768, 64
NCORES = 8
NIC = DI // 128          # 6 contraction chunks
NKT = S // 128           # 32 global key tiles per batch
NOWN = NKT // 2          # 16 own key tiles per core
QT = 512                 # query tile width
NQT = S // QT            # 8 query tiles
ORD = [0, 1, 3, 2, 7, 6, 5, 4]       # query-tile processing order
XORD = [0, 1, 3, 2, 7, 6, 5, 4]      # x column-block DMA order
F32 = mybir.dt.float32
BF16 = mybir.dt.bfloat16

_prog_cache = {}


def j0_of(T):
    """First diagonal-region packed key tile for permuted query tile T."""
    return 4 * T if T < 4 else 4 * (T - 4)


def build_program():
    """Build + compile the single SPMD Bass program (identical on all cores)."""
    nc = bacc.Bacc("TRN2", target_bir_lowering=False, debug=False)

    xT = nc.declare_dram_parameter("xT", [DI, S], BF16, isOutput=False)
    wkv = nc.declare_dram_parameter("wkv", [DI, 128], BF16, isOutput=False)
    # [Wq | Wq] duplicated so Q^T appears at partitions 0..63 AND 64..127,
    # feeding the two concurrent K=64 row-group matmuls of a scores pair.
    wq2 = nc.declare_dram_parameter("wq2", [DI, 128], BF16, isOutput=False)
    mdiag = nc.declare_dram_parameter("mdiag", [128, 128], BF16, isOutput=False)
    mpcol = nc.declare_dram_parameter("mpcol", [128, 128], BF16, isOutput=False)
    ident = nc.declare_dram_parameter("ident", [DO, DO], BF16, isOutput=False)
    nd = nc.declare_dram_parameter("nd", [DO + 1, S], F32, isOutput=True)

    with tile.TileContext(nc) as tc:
        with tc.tile_pool(name="consts", bufs=1) as consts, \
             tc.tile_pool(name="xpool", bufs=1) as xpool, \
             tc.tile_pool(name="qkv", bufs=1) as qkv, \
             tc.tile_pool(name="expp", bufs=10) as expp, \
             tc.tile_pool(name="ndst", bufs=2) as ndst, \
             tc.tile_pool(name="ps_sc", bufs=2, space="PSUM") as ps_sc, \
             tc.tile_pool(name="ps_pj", bufs=1, space="PSUM") as ps_pj, \
             tc.tile_pool(name="ps_ctx", bufs=1, space="PSUM") as ps_ctx:

            # ---- HAM warm-up: keep the PE busy from t=0 while DMA lands, so
            # the clock-gate reaches K=8/8 before the real work arrives.
            dum = consts.tile([128, 128], BF16, tag="dum", name="dum")
            nc.vector.memset(dum, 0.0)
            pdum = ps_sc.tile([128, 3 * QT], F32, tag="psc", name="psc")
            for _ in range(40):
                nc.tensor.matmul(pdum[:, 0:128], dum, dum, start=True, stop=True)

            # ---- constant-ish inputs; the first projection's weights go first,
            # then the first x column block, so the PE can start ASAP
            twkv = consts.tile([128, NIC, 128], BF16, tag="twkv", name="twkv")
            twq = consts.tile([128, NIC, 128], BF16, tag="twq", name="twq")
            nc.sync.dma_start(out=twkv, in_=wkv.rearrange("(c p) w -> p c w", p=128))

            # x^T column blocks, ONE [128, 6, 512] DMA each (6KB/partition -
            # large transfers amortize the ~per-DMA completion latency),
            # issued in XORD order
            xb = [xpool.tile([128, NIC, QT], BF16, tag=f"xb_{cb}", name=f"xb_{cb}")
                  for cb in range(NQT)]
            nc.sync.dma_start(
                out=xb[0],
                in_=xT[:, 0:QT].rearrange("(c p) w -> p c w", p=128))

            nc.sync.dma_start(out=twq, in_=wq2.rearrange("(c p) w -> p c w", p=128))
            tmd = consts.tile([128, 128], BF16, tag="tmd", name="tmd")
            tmp = consts.tile([128, 128], BF16, tag="tmp", name="tmp")
            tid = consts.tile([DO, DO], BF16, tag="tid", name="tid")
            nc.sync.dma_start(out=tmd, in_=mdiag[:, :])
            nc.sync.dma_start(out=tmp, in_=mpcol[:, :])
            nc.sync.dma_start(out=tid, in_=ident[:, :])
            zsrc = consts.tile([DO, 1], F32, tag="zsrc", name="zsrc")
            nc.vector.memset(zsrc, 0.0)

            # Dummy exp to pull the ~2.7us ACT table load off the critical path.
            zexp = consts.tile([DO, 1], F32, tag="zexp", name="zexp")
            nc.scalar.activation(zexp, zsrc,
                                 mybir.ActivationFunctionType.Exp, scale=1.0)

            for cb in XORD[1:]:
                nc.sync.dma_start(
                    out=xb[cb],
                    in_=xT[:, cb * QT:(cb + 1) * QT].rearrange(
                        "(c p) w -> p c w", p=128))

            def xc(ic, cb):
                return xb[cb][:, ic, :]

            # ---- projection state (emitted interleaved with attention) ----
            # kp[i]: K^T of key tile 2i at partitions 0..63, tile 2i+1 at 64..127
            kps = [qkv.tile([128, 128], BF16, tag=f"kp_{i}", name=f"kp_{i}")
                   for i in range(NOWN // 2)]
            vts = [qkv.tile([DO, QT], BF16, tag=f"vt_{st}", name=f"vt_{st}") for st in range(4)]
            qts = [qkv.tile([128, QT], BF16, tag=f"qt_{st}", name=f"qt_{st}") for st in range(NQT)]
            # all V1 tiles in one buffer: [128 keys, key tile, 64 V cols + ones]
            v1big = qkv.tile([128, NOWN, DO + 1], BF16, tag="v1big", name="v1big")
            nc.vector.memset(v1big[:, :, DO:DO + 1], 1.0)

            def v1(j):
                return v1big[:, j, :]

            def pass1_units(st):
                """Small PE work units for the K/V projection of own key
                column block st (emitted interleaved with attention)."""
                p1 = ps_pj.tile([128, QT], F32, tag="pspj", name="pspj")
                for ic in range(0, NIC, 2):
                    def mm2(ic=ic, p1=p1):
                        nc.tensor.matmul(p1, twkv[:, ic, :], xc(ic, st),
                                         start=(ic == 0), stop=False)
                        nc.tensor.matmul(p1, twkv[:, ic + 1, :], xc(ic + 1, st),
                                         start=False, stop=(ic + 1 == NIC - 1))
                    yield mm2

                def copies(p1=p1):
                    nc.vector.tensor_copy(vts[st], p1[DO:128, :])
                    for u in range(2):
                        kp = kps[2 * st + u]
                        nc.vector.tensor_copy(kp[0:DO, :],
                                              p1[0:DO, 256 * u:256 * u + 128])
                        nc.vector.tensor_copy(kp[DO:128, :],
                                              p1[0:DO, 256 * u + 128:256 * u + 256])
                yield copies

                def transp():
                    pvq = ps_pj.tile([128, 4, DO], BF16, tag="pspj", name="pspj")
                    for r in range(4):
                        nc.tensor.transpose(pvq[:, r, :],
                                            vts[st][:, r * 128:r * 128 + 128], tid)
                    nc.vector.tensor_copy(v1big[:, 4 * st:4 * st + 4, 0:DO], pvq)
                yield transp

            def pass2_units(st):
                """Q^T (duplicated at partitions 0..63 / 64..127) for block st."""
                p2 = ps_pj.tile([128, QT], F32, tag="pspj", name="pspj")
                for ic in range(0, NIC, 2):
                    def mm2(ic=ic, p2=p2):
                        nc.tensor.matmul(p2, twq[:, ic, :], xc(ic, st),
                                         start=(ic == 0), stop=False)
                        nc.tensor.matmul(p2, twq[:, ic + 1, :], xc(ic + 1, st),
                                         start=False, stop=(ic + 1 == NIC - 1))
                    yield mm2

                def qcopy(p2=p2):
                    nc.vector.tensor_copy(qts[st], p2)
                yield qcopy

            # projection units consumed during each position of ORD
            PROJ = {0: [(1, True), (1, False)],
                    1: [(3, False), (3, True), (2, True)],
                    2: [(2, False)], 3: [(7, False)], 4: [(6, False)],
                    5: [(5, False)], 6: [(4, False)], 7: []}

            exp_scale = float(1.0 / np.sqrt(DO))

            def mm_sc(T, j, w, sp, off):
                """One K=64 scores matmul: key tile j x last w queries of tile
                T, into sp[:, off:off+w]. Row-group from j's parity."""
                kp = kps[j // 2]
                lo = DO * (j % 2)
                nc.tensor.matmul(sp[:, off:off + w], kp[lo:lo + DO, :],
                                 qts[T][lo:lo + DO, QT - w:QT],
                                 start=True, stop=True)

            class CtxDrain:
                """Phase B for a query tile, drained a few matmuls at a time so
                ready ctx work interleaves between the next tile's scores
                groups in the in-order PE queue."""

                def __init__(self, T, ctx_args):
                    self.T = T
                    self.nk = j0_of(T) + 4
                    self.args = ctx_args
                    self.i = 0
                    self.ctxp = ps_ctx.tile([DO + 1, QT], F32, tag="ctxp",
                                            name="ctxp")

                def drain(self, n):
                    while self.i < len(self.args) and n > 0:
                        j, et_ap, qlo, w = self.args[self.i]
                        nc.tensor.matmul(self.ctxp[:, qlo:QT], v1(j),
                                         et_ap[:, 0:w],
                                         start=(j == 0), stop=(j == self.nk - 1))
                        self.i += 1
                        n -= 1

                def finish(self):
                    self.drain(len(self.args))
                    ost = ndst.tile([DO + 1, QT], F32, tag="ost", name="ost")
                    nc.vector.tensor_copy(ost, self.ctxp)
                    nc.sync.dma_start(out=nd[:, self.T * QT:(self.T + 1) * QT],
                                      in_=ost)

            for u in pass1_units(0):
                u()
            for u in pass2_units(0):
                u()
            pending = None  # CtxDrain from the previous iteration
            for pos in range(NQT):
                T = ORD[pos]
                j0 = j0_of(T)
                mask = tmd if T < 4 else tmp
                ctx_args = []   # (j, et_ap, qlo, w) consumed in phase B

                projq = []      # projection units for upcoming column blocks
                for st, is_p1 in PROJ[pos]:
                    projq.extend(pass1_units(st) if is_p1 else pass2_units(st))
                projq.reverse()  # consume with pop()

                def do_proj(n):
                    while projq and n > 0:
                        projq.pop()()
                        n -= 1

                # full key tiles 0..j0-1 in triples sharing one 3-bank PSUM
                # tile and one 1536-col exp
                ngroups = -(-j0 // 3) + 1
                dn = -(-(len(pending.args) if pending else 0) // ngroups)
                j = 0
                while j < j0:
                    cnt = min(3, j0 - j)
                    sp = ps_sc.tile([128, 3 * QT], F32, tag="psc", name="psc")
                    et = expp.tile([128, 3 * QT], BF16, tag="et", name="et")
                    for u in range(cnt):
                        mm_sc(T, j + u, QT, sp, u * QT)
                    nc.scalar.activation(et[:, 0:cnt * QT], sp[:, 0:cnt * QT],
                                         mybir.ActivationFunctionType.Exp,
                                         scale=exp_scale)
                    for u in range(cnt):
                        ctx_args.append((j + u, et[:, u * QT:(u + 1) * QT], 0, QT))
                    if pending is not None:
                        pending.drain(dn)
                    do_proj(1)
                    j += cnt
                # diagonal band: all 4 tiles in ONE 3-bank tile / one exp:
                # r0 [0:512] bank1, r1 [512:896] bank2, r3 [896:1024] bank2,
                # r2 [1024:1280] bank3 (concurrent pairs hit distinct banks).
                sp = ps_sc.tile([128, 3 * QT], F32, tag="psc", name="psc")
                et = expp.tile([128, 3 * QT], BF16, tag="et", name="et")
                mm_sc(T, j0, QT, sp, 0)
                mm_sc(T, j0 + 1, 384, sp, QT)
                mm_sc(T, j0 + 2, 256, sp, 2 * QT)
                mm_sc(T, j0 + 3, 128, sp, QT + 384)
                nc.scalar.activation(et[:, 0:2 * QT + 256], sp[:, 0:2 * QT + 256],
                                     mybir.ActivationFunctionType.Exp,
                                     scale=exp_scale)
                nc.vector.tensor_mul(et[:, 0:128], et[:, 0:128], mask)
                nc.vector.tensor_mul(et[:, QT:QT + 128], et[:, QT:QT + 128], mask)
                nc.vector.tensor_mul(et[:, QT + 384:2 * QT], et[:, QT + 384:2 * QT],
                                     mask)
                nc.vector.tensor_mul(et[:, 2 * QT:2 * QT + 128],
                                     et[:, 2 * QT:2 * QT + 128], mask)
                ctx_args.append((j0, et[:, 0:QT], 0, QT))
                ctx_args.append((j0 + 1, et[:, QT:QT + 384], 128, 384))
                ctx_args.append((j0 + 2, et[:, 2 * QT:2 * QT + 256], 256, 256))
                ctx_args.append((j0 + 3, et[:, QT + 384:2 * QT], 384, 128))

                do_proj(len(projq))
                if pending is not None:
                    pending.finish()
                pending = CtxDrain(T, ctx_args)
            pending.finish()

    nc.compile()
    return nc


def get_program():
    if "nc" not in _prog_cache:
        _prog_cache["nc"] = build_program()
    return _prog_cache["nc"]


def core_perm(parity):
    """Permuted-to-global column index map: own key tiles first, then other."""
    own = [g for g in range(NKT) if g % 2 == parity]
    other = [g for g in range(NKT) if g % 2 != parity]
    return np.concatenate([np.arange(g * 128, (g + 1) * 128) for g in own + other])


def _to_bf16(a):
    from concourse import mybir as _mybir
    return np.ascontiguousarray(a.astype(_mybir.dt.np(_mybir.dt.bfloat16)))


def make_in_maps(x, Wq, Wk, Wv):
    x = np.asarray(x, dtype=np.float32)
    Wq = np.asarray(Wq, dtype=np.float32)
    Wk = np.asarray(Wk, dtype=np.float32)
    Wv = np.asarray(Wv, dtype=np.float32)
    wkv = _to_bf16(np.concatenate([Wk, Wv], axis=1))
    wq2 = _to_bf16(np.concatenate([Wq, Wq], axis=1))
    mdiag = _to_bf16(np.triu(np.ones((128, 128), dtype=np.float32)))
    ident = _to_bf16(np.eye(DO, dtype=np.float32))
    in_maps = []
    perms = []
    for c in range(NCORES):
        b, par = c // 2, c % 2
        perm = core_perm(par)
        perms.append(perm)
        xTp = _to_bf16(np.ascontiguousarray(x[b].T[:, perm]))
        mpcol = _to_bf16(np.full((128, 128), 1.0 - par, dtype=np.float32))
        in_maps.append({
            "xT": xTp, "wkv": wkv, "wq2": wq2,
            "mdiag": mdiag, "mpcol": mpcol, "ident": ident,
        })
    return in_maps, perms


def combine(results, perms):
    out = np.empty((B, S, DO), dtype=np.float32)
    for b in range(B):
        num = np.zeros((DO, S), dtype=np.float64)
        den = np.zeros((S,), dtype=np.float64)
        for c in (2 * b, 2 * b + 1):
            nd_c = results[c]["nd"].astype(np.float64)
            inv = np.empty(S, dtype=np.int64)
            inv[perms[c]] = np.arange(S)
            nd_g = nd_c[:, inv]
            num += nd_g[:DO]
            den += nd_g[DO]
        out[b] = (num / den).T.astype(np.float32)
    return out


def kernel(x, Wq, Wk, Wv):
    nc = get_program()
    in_maps, perms = make_in_maps(x, Wq, Wk, Wv)
    res = run_bass_kernel_spmd(nc, in_maps, list(range(NCORES)))
    return combine(res.results, perms)
